# revision 39
# baseline (speedup 1.0000x reference)
"""DualAttention (cross+self bidirectional attention, 2 streams) on 8 TRN2 cores.

Sharding: data-parallel over batch (4) x tensor-parallel over heads (2 groups
of 8 heads). Core c handles batch c//2, head-group c%2. Each core computes its
head-group's slice of all 6 input projections, RoPE, the 4 attention combos,
and a partial output projection; the host sums the two partial out-projections
per batch and adds the output bias.

Device-side structure:
  - q/k projections computed TRANSPOSED ([e_out, s]) in head-contiguous row
    layout (chunk g rows = [head 2g d0..63 | head 2g+1 d0..63], rot dims
    d0..31). RoPE = fixed 128x128 permutation matmul (zero rows for the pass
    dims) + elementwise combines with cos/sin tiles extended by ones/zeros on
    pass rows. Scores contract the full 64 head dims in ONE matmul per head
    (tile_position rows 0/64), streaming half the moving columns of a
    rot-split scheme.
  - keys are host-packed (masked keys dropped, padded to SKP): masking becomes
    a -30000 per-partition exp bias on pad rows and attention shrinks ~2x.
  - unnormalized softmax: U^T = exp(scale*S^T + bias). V carries a fused
    64-wide ones BLOCK per head: each head's AV matmul is a full [128,128]
    stationary (dims at parts 0..63, the row-sum r replicated across parts
    64..127 for free -- no separate ones-matmuls, no replication matmul).
    1/r comes from two partition-shifted reciprocal_approx_fast ops straight
    out of PSUM, and O^T is normalized by partition-shifted DVE multiplies
    (odd-head dims read from parts 0..63, written to 64..127).
  - x/W/q/k/U/V/att/Wo/outputs in bf16 (psum accumulation f32, cos/sin applied
    in bf16 anyway); host sums the two partial projections in f32. DMAs are
    issued on the HWDGE queues in exact need order, per-slice, so the PE
    starts ~5us in; rope combines and att accumulation ride the Pool engine.
  - emission is generator-woven so projection/attention/out-projection work
    interleaves in every engine's instruction stream (phases overlap).
"""

import numpy as np

B, S, E, H = 4, 1024, 1024, 16
D, R = 64, 32
HG, EG = 8, 512  # heads / e-columns per head-group
P = 128
KE = E // P  # contraction chunks of a projection
SCALE = D ** -0.5
NCORES = 8
MASK_BIAS = -30000.0

_PROG_CACHE = {}


def _fslices(SF):
    """Even 128-aligned split of SF into ceil(SF/512) chunks (each >=256)."""
    n = (SF + 511) // 512
    step = ((SF // n + 127) // 128) * 128
    out = []
    lo = 0
    while lo < SF:
        hi = min(lo + step, SF)
        out.append((lo, hi))
        lo = hi
    return out


def _build_program(SKP, reps=1, parts="pako"):
    key = (SKP, reps, parts)
    if key in _PROG_CACHE:
        return _PROG_CACHE[key]

    import concourse.bass as bass
    import concourse.tile as tile
    from concourse import bacc, mybir
    from contextlib import ExitStack

    f32 = mybir.dt.float32
    f32r = mybir.dt.float32r
    bf16 = mybir.dt.bfloat16
    NKC = SKP // P
    ts = bass.ts

    nc = bacc.Bacc("TRN2", target_bir_lowering=False, debug=False, num_devices=NCORES)

    def din(name, shape, dt):
        return nc.dram_tensor(name, list(shape), dt, kind="ExternalInput").ap()

    a = {}
    a["x1T"] = din("x1T", (E, S), bf16)
    a["x2T"] = din("x2T", (E, S), bf16)
    a["x1p"] = din("x1p", (E, SKP), bf16)
    a["x2p"] = din("x2p", (E, SKP), bf16)
    for n in ("q1", "q2", "k1", "k2", "v1", "v2"):
        a["W" + n] = din("W" + n, (E, EG), bf16)
    a["Wo1"] = din("Wo1", (EG, S), bf16)
    a["Wo2"] = din("Wo2", (EG, S), bf16)
    for n in ("q1", "q2", "k1", "k2"):
        a["b" + n] = din("b" + n, (P, 4), f32)
    a["bv1"] = din("bv1", (P, EG), f32)
    a["bv2"] = din("bv2", (P, EG), f32)
    for n in ("cq1", "sq1", "cq2", "sq2"):
        a[n] = din(n, (P, S), bf16)
    for n in ("ck1", "sk1", "ck2", "sk2"):
        a[n] = din(n, (P, SKP), bf16)
    a["mb1"] = din("mb1", (P, NKC), f32)
    a["mb2"] = din("mb2", (P, NKC), f32)
    a["perm"] = din("perm", (P, P), bf16)
    out1 = nc.dram_tensor("o1", [S, E], bf16, kind="ExternalOutput").ap()
    out2 = nc.dram_tensor("o2", [S, E], bf16, kind="ExternalOutput").ap()

    Exp = mybir.ActivationFunctionType.Exp

    def emit(tc):
        with ExitStack() as ctx:
            consts = ctx.enter_context(tc.tile_pool(name="consts", bufs=1))
            xpool = ctx.enter_context(tc.tile_pool(name="xpool", bufs=2))
            xpp = ctx.enter_context(tc.tile_pool(name="xpp", bufs=2))
            wpool = ctx.enter_context(tc.tile_pool(name="wpool", bufs=8))
            qkv = ctx.enter_context(tc.tile_pool(name="qkv", bufs=1))
            attp = ctx.enter_context(tc.tile_pool(name="attp", bufs=1))
            upool = ctx.enter_context(tc.tile_pool(name="upool", bufs=3))
            sm = ctx.enter_context(tc.tile_pool(name="sm", bufs=3))
            outp = ctx.enter_context(tc.tile_pool(name="outp", bufs=2))
            # PSUM (8 banks): st 2x[P,1024](4) + shared pj/pp/op 2x[P,512]
            # (2) + oAB 1x[P,1024] (2)
            stps = ctx.enter_context(tc.tile_pool(name="stps", bufs=2, space="PSUM"))
            pjps = ctx.enter_context(tc.tile_pool(name="pjps", bufs=2, space="PSUM"))
            ops_ = ctx.enter_context(tc.tile_pool(name="ops", bufs=1, space="PSUM"))

            # All DMA triggers ride SP: HWDGE trigger holds the issuing
            # engine's SEQ ~630ns (plus any unsatisfied input waits), which
            # on Act/DVE would stall exp/normalize dispatch.
            def next_dma():
                return nc.sync

            # consts ride the Act/DVE HWDGE queues: both SEQs are idle at
            # t=0 (first exp ~30us in), keeping SP free for the critical
            # w/x loads.
            cdmas = [nc.scalar]
            cctr = [0]

            def next_cdma():
                e = cdmas[cctr[0] % len(cdmas)]
                cctr[0] += 1
                return e

            def cload(name, shape, dt, eng=None):
                t = consts.tile(list(shape), dt, tag=name)
                (eng or nc.gpsimd).dma_start(t[:], a[name])
                return t

            # tiny consts on the gpsimd queue (SWDGE issue is slow ~1us/DMA,
            # so only small, early-needed tensors go here)
            perm_t = cload("perm", (P, P), bf16)
            bqk = {n: cload("b" + n, (P, 4), f32) for n in ("q1", "q2", "k1", "k2")}
            mb = {1: cload("mb1", (P, NKC), f32), 2: cload("mb2", (P, NKC), f32)}
            cq = {n: cload(n, (P, S), bf16, next_cdma()) for n in ("cq1", "sq1")}
            ck = {n: cload(n, (P, SKP), bf16, next_cdma()) for n in ("ck2", "sk2")}
            bv = {"v2": cload("bv2", (P, EG), f32, next_cdma())}
            for n in ("ck1", "sk1"):
                ck[n] = cload(n, (P, SKP), bf16, next_cdma())
            bv["v1"] = cload("bv1", (P, EG), f32, next_cdma())
            for n in ("cq2", "sq2"):
                cq[n] = cload(n, (P, S), bf16, next_cdma())

            def body(_=None):
                # ---------- generators (work-unit granularity) ----------
                def alloc_x(pool, slot_tag, SF, nm):
                    """Per-slice tiles: each DMA is its own dependency unit."""
                    tiles = []
                    for i, (lo, hi) in enumerate(_fslices(SF)):
                        t = pool.tile([P, KE, hi - lo], bf16,
                                      tag=f"{slot_tag}{i}", name=f"{nm}_{i}")
                        tiles.append((lo, hi, t))
                    return tiles

                def issue_x(tiles, ap, idx=None, half=None):
                    """Each ko-half of each slice is its own DMA (deps are
                    region-precise, so matmuls over ko 0..3 start as soon as
                    the first half lands)."""
                    src = ap.rearrange("(ko p) s -> p ko s", p=P)
                    for i, (lo, hi, t) in enumerate(tiles):
                        if idx is None or i == idx:
                            for h in (0, 1) if half is None else (half,):
                                next_dma().dma_start(
                                    t[:, 4 * h : 4 * h + 4, :],
                                    src[:, 4 * h : 4 * h + 4, lo:hi],
                                )

                def load_x(pool, slot_tag, ap, SF, nm):
                    tiles = alloc_x(pool, slot_tag, SF, nm)
                    issue_x(tiles, ap)
                    return tiles

                def alloc_w_halves(wname, dt_, SFW):
                    ko = a[wname].shape[0] // P // 2  # k-chunks per half
                    return [
                        wpool.tile([P, ko, SFW], dt_, tag="w", name="wt")
                        for _ in range(2)
                    ]

                def issue_w(tiles, wname, h):
                    next_dma().dma_start(
                        tiles[h][:],
                        a[wname].rearrange("(h ko p) m -> h p ko m", h=2, p=P)[h],
                    )

                def load_w_halves(wname, dt_, SFW):
                    tiles = alloc_w_halves(wname, dt_, SFW)
                    issue_w(tiles, wname, 0)
                    issue_w(tiles, wname, 1)
                    return tiles

                def proj_qk(dst, x_ts, wname, SF, cos_t, sin_t, w_ts=None,
                            order="mi"):
                    """Generator: dst [P, 4, SF] bf16; yields per unit.
                    order="mi": (mi, slice) iteration (k-style: chunk gp
                    complete asap). order="slice": (slice, mi) iteration
                    (q-style: all chunks of sq-half 0 first)."""
                    if w_ts is None:
                        w_ts = load_w_halves("W" + wname, bf16, EG)
                    if order == "mi":
                        units = [(mi, s) for mi in range(4)
                                 for s in range(len(x_ts))]
                    else:
                        units = [(mi, s) for s in range(len(x_ts))
                                 for mi in range(4)]
                    for mi, s in units:
                        lo, hi, x_t = x_ts[s]
                        if True:
                            # ps is dead after the bias-add; the rope perm
                            # matmul reuses it, so one psum tile per unit
                            # and pjps double-buffers across units.
                            ps = pjps.tile([P, 512], f32, tag="pj",
                                           name="pj")
                            pp = ps
                            for ki in range(KE):
                                nc.tensor.matmul(
                                    ps[:, : hi - lo],
                                    w_ts[ki // 4][:, ki % 4, ts(mi, P)],
                                    x_t[:, ki, :],
                                    start=(ki == 0),
                                    stop=(ki == KE - 1),
                                )
                            nc.vector.tensor_scalar_add(
                                dst[:, mi, lo:hi],
                                ps[:, : hi - lo],
                                bqk[wname][:, mi : mi + 1],
                            )
                            # rope: every chunk has rot rows (0-31, 64-95);
                            # pass rows ride through via cos=1/sin=0.
                            nc.tensor.matmul(
                                pp[:, : hi - lo], perm_t[:], dst[:, mi, lo:hi],
                                start=True, stop=True,
                            )
                            tmp = sm.tile([P, 512], f32, tag="tmp", name="tmp")
                            nc.vector.tensor_mul(
                                tmp[:, : hi - lo], pp[:, : hi - lo],
                                sin_t[:, lo:hi],
                            )
                            # SBUF-only combines on the Pool engine
                            nc.gpsimd.tensor_mul(
                                dst[:, mi, lo:hi], dst[:, mi, lo:hi],
                                cos_t[:, lo:hi],
                            )
                            nc.gpsimd.tensor_add(
                                dst[:, mi, lo:hi], dst[:, mi, lo:hi],
                                tmp[:, : hi - lo],
                            )
                            yield

                def proj_v(dst, xp_ts, wname, w_ts=None):
                    """Generator: dst [P, NKC, 8, 128] bf16 (64-wide ones
                    block fused at cols 64:128); yields per mi."""
                    if w_ts is None:
                        w_ts = load_w_halves("W" + wname, bf16, EG)
                    d4 = dst[:].rearrange("p k (h e) -> p k h e", h=8)
                    nc.gpsimd.memset(d4[:, :, :, 64:128], 1.0)
                    for mi in range(NKC):
                        lo, hi, xp_t = next(
                            s for s in xp_ts if s[0] <= P * mi < s[1]
                        )
                        ps = pjps.tile([P, 512], f32, tag="pj", name="pjv")
                        for ki in range(KE):
                            nc.tensor.matmul(
                                ps[:],
                                xp_t[:, ki, P * mi - lo : P * mi - lo + P],
                                w_ts[ki // 4][:, ki % 4, :],
                                start=(ki == 0),
                                stop=(ki == KE - 1),
                            )
                        nc.vector.tensor_add(
                            d4[:, mi, :, 0:64],
                            ps[:].rearrange("p (h e) -> p h e", h=8),
                            bv[wname][:].rearrange("p (h e) -> p h e", h=8),
                        )
                        yield

                def attention(att_t, first, qT, kT, v_t, mbias):
                    """Generator: yields per (sq, gp, ki) and per normalize."""
                    do_k = "k" in parts
                    v4 = v_t[:].rearrange("p k (h e) -> p k h e", h=8)
                    for sq in range(2):
                        for gp in range(4):  # head pair = chunk index
                            hA, hB = 2 * gp, 2 * gp + 1
                            if do_k:
                                oAB = ops_.tile([P, 1024], f32, tag="o", name="oAB")
                                oA = oAB[:, 0:512]
                                oB = oAB[:, 512:1024]
                            for ki in range(NKC):
                                st = stps.tile([P, 1024], f32, tag="st", name="st")
                                nc.tensor.matmul(
                                    st[:, 0:512],
                                    kT[0:64, gp, ts(ki, P)],
                                    qT[0:64, gp, ts(sq, 512)],
                                    start=True, stop=True,
                                )
                                nc.tensor.matmul(
                                    st[:, 512:1024],
                                    kT[64:128, gp, ts(ki, P)],
                                    qT[64:128, gp, ts(sq, 512)],
                                    start=True, stop=True,
                                )
                                ut = upool.tile([P, 1024], bf16, tag="ut", name="ut")
                                nc.scalar.activation(
                                    ut[:], st[:], Exp,
                                    bias=mbias[:, ki : ki + 1], scale=SCALE,
                                )
                                if do_k:
                                    # head A: dims at parts 0..63, row-sum rA
                                    # replicated at parts 64..127 (ones block)
                                    nc.tensor.matmul(
                                        oA,
                                        v4[:, ki, hA, 0:128],
                                        ut[:, 0:512],
                                        start=(ki == 0),
                                        stop=(ki == NKC - 1),
                                    )
                                    # head B: same layout into oB
                                    nc.tensor.matmul(
                                        oB,
                                        v4[:, ki, hB, 0:128],
                                        ut[:, 512:1024],
                                        start=(ki == 0),
                                        stop=(ki == NKC - 1),
                                    )
                                yield
                            if not do_k:
                                continue
                            # normalize: ONE full-partition reciprocal over
                            # the whole [P,1024] AV psum (r is replicated at
                            # parts 64..127 by the ones block; parts 0..63
                            # compute unused 1/dims). Custom-DVE ops require
                            # partition offset 0 on hw; the regular muls
                            # below do the partition shifts.
                            rrs = sm.tile([P, 1024], f32, tag="rrs", name="rrs")
                            nc.vector.reciprocal_approx_fast(rrs[:], oAB[:])
                            dst = att_t[:, gp, ts(sq, 512)]
                            if first:
                                nc.vector.tensor_mul(
                                    dst[0:64], oA[0:64, :], rrs[64:128, 0:512]
                                )
                                nc.vector.tensor_mul(
                                    dst[64:128], oB[0:64, :], rrs[64:128, 512:1024]
                                )
                            else:
                                tmpo = sm.tile([P, 512], bf16, tag="tmpo", name="tm")
                                nc.vector.tensor_mul(
                                    tmpo[0:64, :], oA[0:64, :], rrs[64:128, 0:512]
                                )
                                nc.vector.tensor_mul(
                                    tmpo[64:128, :], oB[0:64, :], rrs[64:128, 512:1024]
                                )
                                nc.gpsimd.tensor_add(dst, dst, tmpo[:])
                            yield

                def outproj(att_t, wo_name, out_ap, wo_ts=None,
                            si_lo=0, si_hi=8, st_psum=False, dma=None):
                    """Generator: yields per si. st_psum carves psum from the
                    stps pool (2-unit pipelining) -- only safe once attention
                    has drained. dma: engine for the output trigger (op1 on
                    SP resolves mid-c2; op2 on Act after exps end, so SP
                    never blocks the next iteration's input loads)."""
                    if wo_ts is None:
                        wo_ts = load_w_halves(wo_name, bf16, S)
                    for si in range(si_lo, si_hi):
                        osb = outp.tile([P, 1024], bf16, tag="ob", name="ob")
                        if st_psum:
                            t = stps.tile([P, 1024], f32, tag="st", name="opst")
                        for fb in range(2):
                            if st_psum:
                                ps = t[:, 512 * fb : 512 * fb + 512]
                            else:
                                ps = pjps.tile([P, 512], f32, tag="pj",
                                               name="pjo")
                            for ei in range(4):
                                nc.tensor.matmul(
                                    ps[:],
                                    att_t[:, ei, ts(si, P)],
                                    wo_ts[ei // 2][:, ei % 2, 512 * fb : 512 * fb + 512],
                                    start=(ei == 0),
                                    stop=(ei == 3),
                                )
                            # (GPSIMD cannot read PSUM on hw -- keep on DVE)
                            nc.vector.tensor_copy(
                                osb[:, 512 * fb : 512 * fb + 512], ps
                            )
                        (dma or nc.sync).dma_start(out_ap[ts(si, P), :], osb[:])
                        yield

                def drain(*gens):
                    for g in gens:
                        for _ in g:
                            pass

                def drain_n(g, n):
                    for _ in range(n):
                        try:
                            next(g)
                        except StopIteration:
                            return False
                    return True

                def weave(mains, sides=(), ratio=3, main_cap=None):
                    """Pull `ratio` units per round from each main gen, then
                    ONE unit from the head of the side chain (sides drain
                    sequentially -- this keeps per-pool slot demand bounded).
                    sides: [(gen, cap), ...]; cap limits units pulled here.
                    main_cap bounds total main units pulled (None = drain)."""
                    mains = [[g, False] for g in mains]
                    chain = [[g, cap] for g, cap in sides]
                    pulled = 0
                    while True:
                        alive = False
                        for rec in mains:
                            if rec[1]:
                                continue
                            for _ in range(ratio):
                                if main_cap is not None and pulled >= main_cap:
                                    break
                                try:
                                    next(rec[0])
                                    pulled += 1
                                    alive = True
                                except StopIteration:
                                    rec[1] = True
                                    break
                        while chain:
                            g, cap = chain[0]
                            if cap <= 0:
                                chain.pop(0)
                                continue
                            try:
                                next(g)
                                chain[0][1] -= 1
                            except StopIteration:
                                chain.pop(0)
                                continue
                            break
                        if not alive:
                            break

                # ---------- tiles ----------
                qT1 = qkv.tile([P, 4, S], bf16, tag="qT1")
                qT2 = qkv.tile([P, 4, S], bf16, tag="qT2")
                kT1 = qkv.tile([P, 4, SKP], bf16, tag="kT1")
                kT2 = qkv.tile([P, 4, SKP], bf16, tag="kT2")
                v1 = qkv.tile([P, NKC, 8 * 128], bf16, tag="v1")
                v2 = qkv.tile([P, NKC, 8 * 128], bf16, tag="v2")
                att1 = attp.tile([P, 4, S], bf16, tag="att1")
                att2 = attp.tile([P, 4, S], bf16, tag="att2")

                # ---------- schedule ----------
                # hoisted loads on the SP HWDGE queue, in exact need order
                # (the sim models one shared DMA engine, so order matters)
                w_q1 = alloc_w_halves("Wq1", bf16, EG)
                issue_w(w_q1, "Wq1", 0)
                x1t = alloc_x(xpool, "x", S, "x1t")
                issue_x(x1t, a["x1T"], 0)
                issue_w(w_q1, "Wq1", 1)
                w_k2 = load_w_halves("Wk2", bf16, EG)
                xp2 = alloc_x(xpp, "xp", SKP, "xp2")
                issue_x(xp2, a["x2p"], 0)
                w_v2 = load_w_halves("Wv2", bf16, EG)
                issue_x(xp2, a["x2p"], 1)
                issue_x(x1t, a["x1T"], 1)
                xp1 = load_x(xpp, "xp", a["x1p"], SKP, "xp1")
                x2t = load_x(xpool, "x", a["x2T"], S, "x2t")

                do_a = "a" in parts
                g_q1 = proj_qk(qT1, x1t, "q1", S, cq["cq1"], cq["sq1"], w_q1,
                               order="slice")
                g_k2 = proj_qk(kT2, xp2, "k2", SKP, ck["ck2"], ck["sk2"],
                               w_k2)
                g_v2 = proj_v(v2, xp2, "v2", w_v2)
                g_k1 = proj_qk(kT1, xp1, "k1", SKP, ck["ck1"], ck["sk1"])
                g_v1 = proj_v(v1, xp1, "v1")
                g_q2 = proj_qk(qT2, x2t, "q2", S, cq["cq2"], cq["sq2"],
                               order="slice")
                if not do_a:
                    drain(g_q1, g_k2, g_v2, g_k1, g_v1, g_q2)
                    z = outp.tile([P, 1024], bf16, tag="ob")
                    nc.vector.memset(z[:], 0.0)
                    nc.sync.dma_start(out1[0:P, :], z[:])
                    nc.sync.dma_start(out2[0:P, :], z[:])
                    return

                # combo unit counts: per sq-half 4*(NKC+1) units
                half_units = 4 * (NKC + 1) if "k" in parts else 4 * NKC
                c0 = attention(att1, True, qT1, kT2, v2, mb[2])
                # --- c0 sq0: pre-projections woven at dependency
                # granularity so the Act exp chain starts ~15us in. Group g
                # needs q1 chunk (sq0,g) + k2 chunk g; AV ki needs v2[ki].
                drain_n(g_k2, 2)
                drain_n(g_q1, 1)
                drain_n(g_v2, 1)
                for g in range(4):
                    for ki in range(NKC):
                        drain_n(c0, 1)
                        # interleave next-group/next-ki projection deps
                        # between attention units so proj stalls overlap
                        # Act-gated waits
                        if g == 0:
                            if ki < NKC - 1:
                                drain_n(g_v2, 1)
                            else:
                                drain_n(g_k2, 1)
                        elif g < 3:
                            if ki == 1 or ki == 2:
                                drain_n(g_k2, 1)
                            elif ki == 3:
                                drain_n(g_q1, 1)
                    drain_n(c0, 1)  # normalize
                    if g == 0:
                        drain_n(g_k2, 1)
                        drain_n(g_q1, 1)
                drain(g_v2)
                # --- c0 sq1: group g's q1 chunk (g, sq1) MUST be pulled
                # before its first score (a score emitted before its writer
                # gets no dep and reads garbage). k1/v1 units fill between.
                for g in range(4):
                    drain_n(g_q1, 1)
                    for ki in range(NKC):
                        drain_n(c0, 1)
                        if ki < 2:
                            drain_n(g_k1, 1)
                        elif ki == 2 or (g == 0 and ki == 3):
                            drain_n(g_v1, 1)
                    drain_n(c0, 1)  # normalize
                drain(c0, g_q1, g_k1, g_v1)

                c1 = attention(att1, False, qT1, kT1, v1, mb[1])
                c2 = attention(att2, True, qT2, kT1, v1, mb[1])
                c3 = attention(att2, False, qT2, kT2, v2, mb[2])
                if "o" in parts:
                    wo1_ts = load_w_halves("Wo1", bf16, S)
                    wo2_ts = load_w_halves("Wo2", bf16, S)
                    op1 = outproj(att1, "Wo1", out1, wo1_ts, dma=nc.sync)
                    op2a = outproj(att2, "Wo2", out2, wo2_ts, 0, 4,
                                   dma=nc.scalar)
                    op2b = outproj(att2, "Wo2", out2, wo2_ts, 4, 8,
                                   st_psum=True, dma=nc.scalar)
                    # q2 proj fills c1; op1 needs att1 sq0 (ready after c1's
                    # first half). Spread op filler thin (Act's exp deficit
                    # is ~1us per group of 6 mains) so every combo keeps
                    # some PE filler to the end.
                    weave([c1], sides=[(g_q2, 99)], ratio=3,
                          main_cap=half_units)
                    weave([c1], sides=[(g_q2, 99), (op1, 3)], ratio=3)
                    drain(g_q2)
                    weave([c2], sides=[(op1, 4)], ratio=6)
                    weave([c3], sides=[(op1, 99)], ratio=6,
                          main_cap=half_units)
                    drain(op1)
                    # op2 si<4 needs att2 sq0 complete; si>=4 needs all
                    weave([c3], sides=[(op2a, 99)], ratio=5)
                    drain(op2a)
                    drain(op2b)
                else:
                    drain(g_q2, c1, c2, c3)
                    z = outp.tile([P, 1024], bf16, tag="ob")
                    nc.vector.memset(z[:], 0.0)
                    nc.sync.dma_start(out1[0:P, :], z[:])
                    nc.sync.dma_start(out2[0:P, :], z[:])

            if reps > 1:
                with tc.For_i(
                    0, reps, 1,
                    hint_engines=(
                        mybir.EngineType.PE,
                        mybir.EngineType.Activation,
                        mybir.EngineType.DVE,
                        mybir.EngineType.SP,
                        mybir.EngineType.Pool,
                    ),
                ):
                    body()
            else:
                body()

    with tile.TileContext(nc) as tc:
        emit(tc)
    nc.compile()
    _PROG_CACHE[key] = nc
    return nc


def _prep_inputs(inputs):
    """Host-side sharding/packing. Returns (in_maps, SKP, bo1, bo2)."""
    f32 = np.float32
    x1 = np.asarray(inputs["x1"], f32)
    x2 = np.asarray(inputs["x2"], f32)
    m1 = np.asarray(inputs["x1_padding_mask"]).astype(np.int64)
    m2 = np.asarray(inputs["x2_padding_mask"]).astype(np.int64)
    cos1 = np.asarray(inputs["cos1"], f32).reshape(S, R)
    sin1 = np.asarray(inputs["sin1"], f32).reshape(S, R)
    cos2 = np.asarray(inputs["cos2"], f32).reshape(S, R)
    sin2 = np.asarray(inputs["sin2"], f32).reshape(S, R)

    idx1 = [np.nonzero(m1[b])[0] for b in range(B)]
    idx2 = [np.nonzero(m2[b])[0] for b in range(B)]
    maxn = max([len(i) for i in idx1] + [len(i) for i in idx2] + [1])
    SKP = ((maxn + P - 1) // P) * P
    NKC = SKP // P

    import ml_dtypes

    bf16 = ml_dtypes.bfloat16

    gw = []
    for g in range(2):
        rows = slice(g * EG, (g + 1) * EG)
        d = {}
        for n in ("q1", "q2", "k1", "k2"):
            W = np.asarray(inputs["W" + n], f32)[rows]
            bb = np.asarray(inputs["b" + n], f32)[rows]
            d["W" + n] = np.ascontiguousarray(W.T).astype(bf16)
            d["b" + n] = np.ascontiguousarray(bb.reshape(4, P).T)
        for n in ("v1", "v2"):
            W = np.asarray(inputs["W" + n], f32)[rows]
            bb = np.asarray(inputs["b" + n], f32)[rows]
            d["W" + n] = np.ascontiguousarray(W.T).astype(bf16)
            d["b" + n] = np.ascontiguousarray(np.broadcast_to(bb[None, :], (P, EG)))
        for n in ("o1", "o2"):
            W = np.asarray(inputs["W" + n], f32)[:, rows]
            d["W" + n] = np.ascontiguousarray(W.T).astype(bf16)
        gw.append(d)

    def ext_q(v, fill):
        # [S, R] -> [P, S]: rows [v.T(32) | fill(32)] x2 (pass rows ride
        # through rope via cos=1/sin=0)
        blk = np.concatenate(
            [v.T, np.full((D - R, v.shape[0]), fill, f32)], axis=0
        )
        return np.tile(blk, (2, 1))

    cq = {
        "cq1": ext_q(cos1, 1.0).astype(bf16),
        "sq1": ext_q(sin1, 0.0).astype(bf16),
        "cq2": ext_q(cos2, 1.0).astype(bf16),
        "sq2": ext_q(sin2, 0.0).astype(bf16),
    }
    pm = np.zeros((P, P), f32)
    for blk in range(2):
        o = 64 * blk
        for i in range(16):
            pm[o + 16 + i, o + i] = -1.0
            pm[o + i, o + 16 + i] = 1.0

    in_maps = []
    for c in range(NCORES):
        b, g = c // 2, c % 2
        m = {}
        m["x1T"] = np.ascontiguousarray(x1[b].T).astype(bf16)
        m["x2T"] = np.ascontiguousarray(x2[b].T).astype(bf16)
        for which, xb, idx, cos, sin in (
            (1, x1[b], idx1[b], cos1, sin1),
            (2, x2[b], idx2[b], cos2, sin2),
        ):
            n = len(idx)
            xp = np.zeros((SKP, E), f32)
            xp[:n] = xb[idx]
            m[f"x{which}p"] = np.ascontiguousarray(xp.T).astype(bf16)
            ckk = np.zeros((R, SKP), f32)
            skk = np.zeros((R, SKP), f32)
            ckk[:, :n] = cos.T[:, idx]
            skk[:, :n] = sin.T[:, idx]
            cblk = np.concatenate([ckk, np.ones((D - R, SKP), f32)], axis=0)
            sblk = np.concatenate([skk, np.zeros((D - R, SKP), f32)], axis=0)
            m[f"ck{which}"] = np.tile(cblk, (2, 1)).astype(bf16)
            m[f"sk{which}"] = np.tile(sblk, (2, 1)).astype(bf16)
            mbv = np.full(SKP, MASK_BIAS, f32)
            mbv[:n] = 0.0
            m[f"mb{which}"] = np.ascontiguousarray(mbv.reshape(NKC, P).T)
        for n in ("q1", "q2", "k1", "k2", "v1", "v2"):
            m["W" + n] = gw[g]["W" + n]
            m["b" + n] = gw[g]["b" + n]
        m["Wo1"] = gw[g]["Wo1"]
        m["Wo2"] = gw[g]["Wo2"]
        m.update(cq)
        m["perm"] = pm.astype(bf16)
        in_maps.append(m)

    bo1 = np.asarray(inputs["bo1"], f32)
    bo2 = np.asarray(inputs["bo2"], f32)
    return in_maps, SKP, bo1, bo2


def kernel(**inputs):
    from concourse.bass_utils import run_bass_kernel_spmd

    in_maps, SKP, bo1, bo2 = _prep_inputs(inputs)
    nc = _build_program(SKP)
    res = run_bass_kernel_spmd(nc, in_maps, core_ids=list(range(NCORES)))
    o1 = np.stack(
        [res.results[2 * b]["o1"].astype(np.float32)
         + res.results[2 * b + 1]["o1"].astype(np.float32) + bo1
         for b in range(B)]
    )
    o2 = np.stack(
        [res.results[2 * b]["o2"].astype(np.float32)
         + res.results[2 * b + 1]["o2"].astype(np.float32) + bo2
         for b in range(B)]
    )
    return o1.astype(np.float32), o2.astype(np.float32)

